# revision 1
# baseline (speedup 1.0000x reference)
"""Multi-head attention (B=2, N=4096, D=512, H=8) on 8 TRN2 NeuronCores.

Sharding: head-parallel. Core d owns head d for both batches:
  - QKV: tensor-parallel slices of w_qkv (per-head 64-dim slices), computed
    from a replicated transposed activation xT = x.T (bf16).
  - Attention: flash-style, scores kept transposed (S.T = k @ q.T per
    128-k-token tile), softmax without max subtraction (scores ~ N(0,1)),
    exp on ScalarE with the 1/sqrt(hd) scale fused in, attn.T @ v via a
    [v | ones] stationary operand so the softmax denominator falls out of
    the same matmul (row 0 of the accumulator).
  - Two half-shard AllToAlls redistribute normalized per-head outputs so
    core d holds all heads for tokens [d*1024, (d+1)*1024); the first A2A
    fires halfway through attention and is fully hidden. A local output
    projection (bias folded in as a K=1 matmul term) produces the slice.
Host side only transposes/casts inputs and concatenates the 8 output slices.

Perf notes (HW ~394us best, ScalarE-bound; chip-level throttle adds up to
~30% run-to-run): PE runs at the sustained 1.2 GHz throttled clock on
this fleet, so scores are row-tiled (the two batches' K=64 matmuls run
concurrently in separate PE row groups) and the QKV projections are
col-tiled across batches. The exp stream on ScalarE (33.5M elems/core,
~276us busy at 92%+ utilization) is the bottleneck; scores+exp are
emitted one k-tile ahead of the accumulation matmuls, qT/v' production is
interleaved into the first attention chunk through PSUM slots shared with
the attention accumulators, and the softmax normalization (spread-
partition reciprocal + DRAM-bounce broadcast) runs entirely off the
PE/ACT streams.
"""

import numpy as np
import ml_dtypes

N_CORES = 8
B, N, D = 2, 4096, 512
H, HD = 8, 64
T = B * N              # 8192 flattened tokens
TS = T // N_CORES      # 1024 tokens output slice per core
SCALE = HD ** -0.5
KC = D // 128          # 4 contraction chunks of the model dim
NKT = N // 128         # 32 k-token tiles per batch
QC = 512               # q-chunk (columns) processed per accumulator
NQC = N // QC          # 4 q-chunks per batch

BF16 = ml_dtypes.bfloat16

_COMPILED = {}


def _patch_tile_drain():
    """The walrus build in this container caps sync waits at 1 per
    instruction (2 for EventSemaphore), but TileContext._drain_and_barrier
    puts every live proc's final wait on a single Drain, which fails
    codegen with 'Too many sync wait commands'. Re-emit those waits as
    individual wait_ge instructions before the drain."""
    import concourse.mybir as mybir
    import concourse.tile as tile
    from concourse.bass_types import SemaphoreHandle
    from concourse.vector_clock import ScopedClock

    if getattr(tile.TileContext, "_drain_patch_installed", False):
        return

    def _drain_and_barrier(self, tick_clock, wait_clock):
        probe = mybir.InstNoOp(name=f"drain-probe-{self.nc.next_id()}", ins=[], outs=[])
        probe.engine = mybir.EngineType.SP
        wait_clock.add_sem_waits(probe, ScopedClock({None: tick_clock.global_clock}))
        waits = probe.sync_info.on_wait if probe.sync_info is not None else []
        for w in waits:
            assert w.wait_mode == "sem-ge-imm", w
            self.nc.sync.wait_ge(SemaphoreHandle(w.ant_name, w.id), w.wait_value)
        self.nc.sync.drain()

        self.nc.all_engine_barrier()
        assert self.sems is not None
        popped = self.nc._tile_sem_poison_stack.pop()
        assert popped is self._sem_poison
        self.nc.clear_and_free_semaphores(list(self.sems.allocated().values()))
        self.nc.all_engine_barrier()

    tile.TileContext._drain_and_barrier = _drain_and_barrier
    tile.TileContext._drain_patch_installed = True


def _patch_multiwait_split():
    """This walrus build rejects instructions with more than one sync wait
    ('Too many sync wait commands'), but Tile's wait assigner can emit
    several waits on one instruction. Post-process the serialized BIR:
    move excess waits onto single-wait EventSemaphore instructions inserted
    just before the owning instruction (same engine => executes in order)."""
    import json

    import concourse.bass as bass

    if getattr(bass.Bass, "_multiwait_patch_installed", False):
        return
    orig = bass.Bass.to_json_bytes

    def to_json_bytes(self, *a, **kw):
        data = json.loads(orig(self, *a, **kw))
        n_split = 0
        for fn in data.get("functions", []):
            for bb in fn.get("blocks", []):
                insts = bb.get("instructions")
                if not insts:
                    continue
                out = []
                for inst in insts:
                    si = inst.get("sync_info")
                    ow = (si or {}).get("on_wait") or []
                    if len(ow) > 1:
                        for i, w in enumerate(ow[:-1]):
                            out.append({
                                "debug": inst.get("debug", 0),
                                "engine": inst["engine"],
                                "ins": [],
                                "outs": [],
                                "name": f"{inst['name']}-esw{i}",
                                "opcode": "EventSemaphore",
                                "sync_info": {"on_update": [], "on_wait": [w]},
                            })
                            n_split += 1
                        si["on_wait"] = [ow[-1]]
                    out.append(inst)
                bb["instructions"] = out
        return json.dumps(data).encode()

    bass.Bass.to_json_bytes = to_json_bytes
    bass.Bass._multiwait_patch_installed = True


def _build():
    from contextlib import ExitStack

    import concourse.bass as bass
    import concourse.mybir as mybir
    import concourse.tile as tile

    _patch_tile_drain()
    _patch_multiwait_split()
    dt = mybir.dt
    nc = bass.Bass(num_devices=N_CORES)

    xT_ext = nc.declare_dram_parameter("xT", [D, T], dt.bfloat16, isOutput=False)
    wqT_ext = nc.declare_dram_parameter("wqT", [D, HD], dt.bfloat16, isOutput=False)
    wkT_ext = nc.declare_dram_parameter("wkT", [D, HD], dt.bfloat16, isOutput=False)
    wvT_ext = nc.declare_dram_parameter("wvT", [D, HD], dt.bfloat16, isOutput=False)
    wpT_ext = nc.declare_dram_parameter("wpT", [D, D], dt.bfloat16, isOutput=False)
    bias_ext = nc.declare_dram_parameter("bias", [1, D], dt.bfloat16, isOutput=False)
    out_ext = nc.declare_dram_parameter("out", [TS, D], dt.float32, isOutput=True)

    with tile.TileContext(nc) as tc, ExitStack() as ctx:
        singles = ctx.enter_context(tc.tile_pool(name="singles", bufs=1))
        dram = ctx.enter_context(tc.tile_pool(name="dram", bufs=2, space="DRAM"))
        cpool = ctx.enter_context(tc.tile_pool(name="cpool", bufs=4))

        # ---------- persistent SBUF ----------
        xT_k = [
            singles.tile([128, T], dt.bfloat16, tag=f"xT{k}", name=f"xT{k}")
            for k in range(KC)
        ]
        wqT_sb = singles.tile([128, KC, HD], dt.bfloat16)
        wkT_sb = singles.tile([128, KC, HD], dt.bfloat16)
        wvT_sb = singles.tile([128, KC, HD], dt.bfloat16)
        wpT_sb = singles.tile([128, KC, D], dt.bfloat16)
        bias_sb = singles.tile([1, D], dt.bfloat16)
        ones_sb = singles.tile([1, 128], dt.bfloat16)
        # qT/kT: partitions 0-63 = batch 0, 64-127 = batch 1; cols = token in batch
        # qT and v are split into small tiles so attention chunks can start
        # before the whole QKV phase finishes.
        qT_t = [
            singles.tile([128, 512], dt.bfloat16, tag=f"qT{i}", name=f"qT{i}")
            for i in range(N // 512)
        ]
        kT_t = [
            singles.tile([128, 512], dt.bfloat16, tag=f"kT{i}", name=f"kT{i}")
            for i in range(N // 512)
        ]
        vp_t = [
            singles.tile([128, 1 + HD], dt.bfloat16, tag=f"vp{t}", name=f"vp{t}")
            for t in range(T // 128)
        ]
        outTall_sb = singles.tile([128, KC, TS], dt.bfloat16)

        # Two half-token A2As: "a" carries the first 512 tokens of every
        # 1024-token shard, "b" the second 512 — so A2A "a" can fire while
        # the second half of attention still runs.
        HTS = TS // 2
        a2a_in_a = dram.tile([N_CORES, HD, HTS], dt.bfloat16)
        a2a_in_b = dram.tile([N_CORES, HD, HTS], dt.bfloat16)
        a2a_out_a = dram.tile([N_CORES, HD, HTS], dt.bfloat16)
        a2a_out_b = dram.tile([N_CORES, HD, HTS], dt.bfloat16)

        for k in range(KC):
            for c in range(4):
                nc.sync.dma_start(
                    out=xT_k[k][:, c * (T // 4):(c + 1) * (T // 4)],
                    in_=xT_ext[k * 128:(k + 1) * 128,
                               c * (T // 4):(c + 1) * (T // 4)],
                )
        for w_sb, w_ext in ((wqT_sb, wqT_ext), (wkT_sb, wkT_ext), (wvT_sb, wvT_ext)):
            nc.sync.dma_start(
                out=w_sb[:], in_=w_ext[:].rearrange("(k p) c -> p k c", p=128)
            )
        nc.sync.dma_start(
            out=wpT_sb[:], in_=wpT_ext[:].rearrange("(k p) c -> p k c", p=128)
        )
        nc.sync.dma_start(out=bias_sb[:], in_=bias_ext[:])
        nc.vector.memset(ones_sb[:], 1.0)
        for t in range(T // 128):
            nc.vector.memset(vp_t[t][:, 0:1], 1.0)

        # ---------- phase 1: kT then qT, both batches col-tiled concurrently --
        # k outermost + 8 persistent accumulators so matmuls on the first
        # xT chunk start as soon as its DMA lands. kT first: attention needs
        # all of kT but only the first qT tile to begin.
        def qk_phase(w_sb, copy_out, pname):
            with tc.tile_pool(name=pname, bufs=1, space="PSUM") as pqk:
                pss = [
                    pqk.tile([128, 512], dt.float32, name=f"{pname}{n}")
                    for n in range(N // 512)
                ]
                for k in range(KC):
                    for n in range(N // 512):
                        nc.tensor.matmul(
                            pss[n][0:64, :],
                            lhsT=w_sb[:, k, :],
                            rhs=xT_k[k][:, n * 512:(n + 1) * 512],
                            start=(k == 0),
                            stop=(k == KC - 1),
                            tile_position=(0, 0),
                        )
                        nc.tensor.matmul(
                            pss[n][64:128, :],
                            lhsT=w_sb[:, k, :],
                            rhs=xT_k[k][:, N + n * 512:N + (n + 1) * 512],
                            start=(k == 0),
                            stop=(k == KC - 1),
                            tile_position=(0, 64),
                        )
                for n in range(N // 512):
                    copy_out(n, pss[n])

        qk_phase(wkT_sb,
                 lambda n, ps: nc.vector.tensor_copy(kT_t[n][:], ps[:]), "pk")

        # ---------- phase 3: attention (both batches per iteration) ----------
        # qT tiles and v' tiles are produced through a single aux PSUM bank,
        # interleaved into the first q-chunk's kt loop so they ride in PE's
        # slack while ScalarE (the bottleneck) streams exps.
        with (
            tc.tile_pool(name="pst", bufs=2, space="PSUM") as pst,
            tc.tile_pool(name="pacc", bufs=4, space="PSUM") as pacc,
        ):
            def produce_qT(n):
                ps = pacc.tile([128, 512], dt.float32, tag="acc", name=f"pq{n}")
                for k in range(KC):
                    nc.tensor.matmul(
                        ps[0:64, :],
                        lhsT=wqT_sb[:, k, :],
                        rhs=xT_k[k][:, n * 512:(n + 1) * 512],
                        start=(k == 0), stop=(k == KC - 1),
                        tile_position=(0, 0),
                    )
                    nc.tensor.matmul(
                        ps[64:128, :],
                        lhsT=wqT_sb[:, k, :],
                        rhs=xT_k[k][:, N + n * 512:N + (n + 1) * 512],
                        start=(k == 0), stop=(k == KC - 1),
                        tile_position=(0, 64),
                    )
                nc.vector.tensor_copy(qT_t[n][:], ps[:])

            def produce_v(t):
                pv = pacc.tile([128, HD], dt.float32, tag="acc", name=f"pv{t}")
                for k in range(KC):
                    nc.tensor.matmul(
                        pv[:],
                        lhsT=xT_k[k][:, t * 128:(t + 1) * 128],
                        rhs=wvT_sb[:, k, :],
                        start=(k == 0), stop=(k == KC - 1),
                    )
                nc.vector.tensor_copy(vp_t[t][:, 1:1 + HD], pv[:])

            produce_qT(0)
            produce_v(0)
            produce_v(NKT)

            qc_order = [q for q in range(NQC) if q % 2 == 0] + \
                       [q for q in range(NQC) if q % 2 == 1]

            def emit_scores_exp(qc, kt):
                st = pst.tile([128, B, QC], dt.float32, tag="st",
                              name=f"st{qc}_{kt}")
                for pair in range(B):
                    pb = pair * 64
                    lhs_k = kT_t[kt // 4][pb:pb + 64,
                                          (kt % 4) * 128:(kt % 4) * 128 + 128]
                    nc.tensor.matmul(
                        st[:, pair, :],
                        lhsT=lhs_k,
                        rhs=qT_t[qc][pb:pb + 64, :],
                        start=True,
                        stop=True,
                        tile_position=(pb, 0),
                    )
                e = cpool.tile([128, B, QC], dt.bfloat16, tag="e", bufs=6,
                               name=f"e{qc}_{kt}")
                nc.scalar.activation(
                    e[:], st[:], mybir.ActivationFunctionType.Exp, scale=SCALE
                )
                return e

            se = emit_scores_exp(qc_order[0], 0)
            for qi, qc in enumerate(qc_order):
                accs = [
                    pacc.tile([1 + HD, QC], dt.float32, tag="acc", name=f"acc{qc}_{p}")
                    for p in range(B)
                ]
                for kt in range(NKT):
                    e = se
                    if kt < NKT - 1:
                        se = emit_scores_exp(qc, kt + 1)
                    elif qi + 1 < len(qc_order):
                        se = emit_scores_exp(qc_order[qi + 1], 0)
                    if qi == 0:
                        # pipeline the remaining qkv production into PE slack
                        if kt < NKT - 1:
                            produce_v(kt + 1)
                            produce_v(NKT + kt + 1)
                        if kt < NQC - 1:
                            produce_qT(kt + 1)
                    for pair in range(B):
                        vkt = vp_t[pair * NKT + kt][:]
                        nc.tensor.matmul(
                            accs[pair][:, :],
                            lhsT=vkt,
                            rhs=e[:, pair, :],
                            start=(kt == 0),
                            stop=(kt == NKT - 1),
                        )
                # normalization front half (DVE only): copy accumulator off
                # PSUM (frees the slot early), reciprocal of the denominator
                # row, cast for the matmul broadcast
                # normalize: copy accumulator off PSUM (frees the slot
                # early); reciprocal of the denominator row spread over 64
                # partitions via a strided DRAM bounce, then partition-
                # broadcast back via DRAM; everything off the PE/ACT streams
                for pair in range(B):
                    acc = accs[pair]
                    accS = cpool.tile([1 + HD, QC], dt.float32, tag="accS",
                                      name=f"accS{qc}_{pair}")
                    nc.vector.tensor_copy(accS[:], acc[:])
                    rdram = dram.tile([1, QC], dt.float32, tag="rdram")
                    nc.sync.dma_start(out=rdram[:], in_=accS[0:1, :])
                    spread = cpool.tile([64, QC // 64], dt.float32, tag="spread")
                    rap = rdram[:]
                    nc.sync.dma_start(
                        out=spread[:],
                        in_=bass.AP(
                            tensor=rap.tensor, offset=rap.offset,
                            ap=[[QC // 64, 64], [1, QC // 64]],
                        ),
                    )
                    rspread = cpool.tile([64, QC // 64], dt.float32, tag="rspread")
                    nc.vector.reciprocal(rspread[:], spread[:])
                    rdram2 = dram.tile([1, QC], dt.float32, tag="rdram2")
                    r2ap = rdram2[:]
                    nc.sync.dma_start(
                        out=bass.AP(
                            tensor=r2ap.tensor, offset=r2ap.offset,
                            ap=[[QC // 64, 64], [1, QC // 64]],
                        ),
                        in_=rspread[:],
                    )
                    bcast = cpool.tile([1 + HD, QC], dt.float32, tag="bcast")
                    nc.sync.dma_start(
                        out=bcast[:],
                        in_=bass.AP(
                            tensor=r2ap.tensor, offset=r2ap.offset,
                            ap=[[0, 1 + HD]] + list(r2ap.ap[1:]),
                        ),
                    )
                    outTn = cpool.tile([1 + HD, QC], dt.bfloat16, tag="outTn",
                                       name=f"oTn{qc}_{pair}")
                    nc.vector.tensor_mul(outTn[:], accS[:], bcast[:])
                    goff = pair * N + qc * QC      # global token offset
                    shard = goff // TS
                    half = a2a_in_a if (goff % TS) < HTS else a2a_in_b
                    nc.sync.dma_start(out=half[shard], in_=outTn[1:1 + HD, :])
                if qi == NQC // 2 - 1:
                    # all first-half shards written -> overlap this A2A with
                    # the remaining attention chunks
                    nc.gpsimd.collective_compute(
                        "AllToAll",
                        mybir.AluOpType.bypass,
                        replica_groups=[list(range(N_CORES))],
                        ins=[a2a_in_a.opt()],
                        outs=[a2a_out_a.opt()],
                    )
                    for k in range(KC):
                        nc.sync.dma_start(
                            out=outTall_sb[:, k, 0:HTS],
                            in_=a2a_out_a[2 * k:2 * k + 2].rearrange(
                                "a d n -> (a d) n"),
                        )

        # ---------- phase 4: output projection on own token slice ----------
        with tc.tile_pool(name="py", bufs=2, space="PSUM") as py:
            def proj_subtile(ts_i):
                yp = py.tile([128, D], dt.float32, name=f"yp{ts_i}", tag="yp")
                for k in range(KC):
                    nc.tensor.matmul(
                        yp[:],
                        lhsT=outTall_sb[:, k, ts_i * 128:(ts_i + 1) * 128],
                        rhs=wpT_sb[:, k, :],
                        start=(k == 0),
                        stop=False,
                    )
                nc.tensor.matmul(
                    yp[:],
                    lhsT=ones_sb[:],
                    rhs=bias_sb[:],
                    start=False,
                    stop=True,
                )
                y_sb = cpool.tile([128, D], dt.float32, tag="y", name=f"y{ts_i}")
                nc.vector.tensor_copy(y_sb[:], yp[:])
                nc.sync.dma_start(
                    out=out_ext[ts_i * 128:(ts_i + 1) * 128, :], in_=y_sb[:]
                )

            # first-half subtiles only need A2A "a" results -> can run while
            # A2A "b" is still in flight
            for ts_i in range(TS // 256):
                proj_subtile(ts_i)

            nc.gpsimd.collective_compute(
                "AllToAll",
                mybir.AluOpType.bypass,
                replica_groups=[list(range(N_CORES))],
                ins=[a2a_in_b.opt()],
                outs=[a2a_out_b.opt()],
            )
            for k in range(KC):
                nc.sync.dma_start(
                    out=outTall_sb[:, k, HTS:TS],
                    in_=a2a_out_b[2 * k:2 * k + 2].rearrange("a d n -> (a d) n"),
                )
            for ts_i in range(TS // 256, TS // 128):
                proj_subtile(ts_i)

    return nc


def _get_nc():
    if "nc" not in _COMPILED:
        _COMPILED["nc"] = _build()
    return _COMPILED["nc"]


def kernel(x, w_qkv, w_proj, b_proj):
    from concourse.bass_utils import run_bass_kernel_spmd

    x = np.asarray(x, dtype=np.float32)
    w_qkv = np.asarray(w_qkv, dtype=np.float32)
    w_proj = np.asarray(w_proj, dtype=np.float32)
    b_proj = np.asarray(b_proj, dtype=np.float32)

    # host-side layout prep (bf16 compute precision on device)
    xT = np.ascontiguousarray(
        x.transpose(2, 0, 1).reshape(D, T)
    ).astype(BF16)
    wpT = np.ascontiguousarray(w_proj.T).astype(BF16)
    bias = b_proj.reshape(1, D).astype(BF16)

    in_maps = []
    for d in range(N_CORES):
        wq = w_qkv[0 * D + d * HD: 0 * D + (d + 1) * HD, :]   # [64, 512]
        wk = w_qkv[1 * D + d * HD: 1 * D + (d + 1) * HD, :]
        wv = w_qkv[2 * D + d * HD: 2 * D + (d + 1) * HD, :]
        in_maps.append({
            "xT": xT,
            "wqT": np.ascontiguousarray(wq.T).astype(BF16),
            "wkT": np.ascontiguousarray(wk.T).astype(BF16),
            "wvT": np.ascontiguousarray(wv.T).astype(BF16),
            "wpT": wpT,
            "bias": bias,
        })

    nc = _get_nc()
    res = run_bass_kernel_spmd(nc, in_maps, core_ids=list(range(N_CORES)))
    y = np.concatenate([res.results[d]["out"] for d in range(N_CORES)], axis=0)
    return y.reshape(B, N, D).astype(np.float32)



# revision 3
# speedup vs baseline: 1.4344x; 1.4344x over previous
"""Multi-head attention (B=2, N=4096, D=512, H=8) on 8 TRN2 NeuronCores.

Sharding: head-parallel. Core d owns head d for both batches:
  - QKV: tensor-parallel slices of w_qkv (per-head 64-dim slices), computed
    from a replicated transposed activation xT = x.T (bf16).
  - Attention: flash-style, scores kept transposed (S.T = k @ q.T per
    128-k-token tile), softmax without max subtraction (scores ~ N(0,1)),
    exp on ScalarE with the 1/sqrt(hd) scale fused in, attn.T @ v via a
    [v | ones] stationary operand so the softmax denominator falls out of
    the same matmul (row 0 of the accumulator).
  - Two half-shard AllToAlls redistribute normalized per-head outputs so
    core d holds all heads for tokens [d*1024, (d+1)*1024); the first A2A
    fires halfway through attention and is fully hidden. A local output
    projection (bias folded in as a K=1 matmul term) produces the slice.
Host side only transposes/casts inputs and concatenates the 8 output slices.

Perf notes (HW ~394us best, ScalarE-bound; chip-level throttle adds up to
~30% run-to-run): PE runs at the sustained 1.2 GHz throttled clock on
this fleet, so scores are row-tiled (the two batches' K=64 matmuls run
concurrently in separate PE row groups) and the QKV projections are
col-tiled across batches. The exp stream on ScalarE (33.5M elems/core,
~276us busy at 92%+ utilization) is the bottleneck; scores+exp are
emitted one k-tile ahead of the accumulation matmuls, qT/v' production is
interleaved into the first attention chunk through PSUM slots shared with
the attention accumulators, and the softmax normalization (spread-
partition reciprocal + DRAM-bounce broadcast) runs entirely off the
PE/ACT streams.
"""

import numpy as np
import ml_dtypes

N_CORES = 8
B, N, D = 2, 4096, 512
H, HD = 8, 64
T = B * N              # 8192 flattened tokens
TS = T // N_CORES      # 1024 tokens output slice per core
SCALE = HD ** -0.5
KC = D // 128          # 4 contraction chunks of the model dim
NKT = N // 128         # 32 k-token tiles per batch
QC = 512               # q-chunk (columns) processed per accumulator
NQC = N // QC          # 4 q-chunks per batch

BF16 = ml_dtypes.bfloat16

_COMPILED = {}


def _patch_tile_drain():
    """The walrus build in this container caps sync waits at 1 per
    instruction (2 for EventSemaphore), but TileContext._drain_and_barrier
    puts every live proc's final wait on a single Drain, which fails
    codegen with 'Too many sync wait commands'. Re-emit those waits as
    individual wait_ge instructions before the drain."""
    import concourse.mybir as mybir
    import concourse.tile as tile
    from concourse.bass_types import SemaphoreHandle
    from concourse.vector_clock import ScopedClock

    if getattr(tile.TileContext, "_drain_patch_installed", False):
        return

    def _drain_and_barrier(self, tick_clock, wait_clock):
        probe = mybir.InstNoOp(name=f"drain-probe-{self.nc.next_id()}", ins=[], outs=[])
        probe.engine = mybir.EngineType.SP
        wait_clock.add_sem_waits(probe, ScopedClock({None: tick_clock.global_clock}))
        waits = probe.sync_info.on_wait if probe.sync_info is not None else []
        for w in waits:
            assert w.wait_mode == "sem-ge-imm", w
            self.nc.sync.wait_ge(SemaphoreHandle(w.ant_name, w.id), w.wait_value)
        self.nc.sync.drain()

        self.nc.all_engine_barrier()
        assert self.sems is not None
        popped = self.nc._tile_sem_poison_stack.pop()
        assert popped is self._sem_poison
        self.nc.clear_and_free_semaphores(list(self.sems.allocated().values()))
        self.nc.all_engine_barrier()

    tile.TileContext._drain_and_barrier = _drain_and_barrier
    tile.TileContext._drain_patch_installed = True


def _patch_multiwait_split():
    """This walrus build rejects instructions with more than one sync wait
    ('Too many sync wait commands'), but Tile's wait assigner can emit
    several waits on one instruction. Post-process the serialized BIR:
    move excess waits onto single-wait EventSemaphore instructions inserted
    just before the owning instruction (same engine => executes in order)."""
    import json

    import concourse.bass as bass

    if getattr(bass.Bass, "_multiwait_patch_installed", False):
        return
    orig = bass.Bass.to_json_bytes

    def to_json_bytes(self, *a, **kw):
        data = json.loads(orig(self, *a, **kw))
        n_split = 0
        for fn in data.get("functions", []):
            for bb in fn.get("blocks", []):
                insts = bb.get("instructions")
                if not insts:
                    continue
                out = []
                for inst in insts:
                    si = inst.get("sync_info")
                    ow = (si or {}).get("on_wait") or []
                    if len(ow) > 1:
                        for i, w in enumerate(ow[:-1]):
                            out.append({
                                "debug": inst.get("debug", 0),
                                "engine": inst["engine"],
                                "ins": [],
                                "outs": [],
                                "name": f"{inst['name']}-esw{i}",
                                "opcode": "EventSemaphore",
                                "sync_info": {"on_update": [], "on_wait": [w]},
                            })
                            n_split += 1
                        si["on_wait"] = [ow[-1]]
                    out.append(inst)
                bb["instructions"] = out
        return json.dumps(data).encode()

    bass.Bass.to_json_bytes = to_json_bytes
    bass.Bass._multiwait_patch_installed = True


def _build():
    from contextlib import ExitStack

    import concourse.bass as bass
    import concourse.mybir as mybir
    import concourse.tile as tile

    _patch_tile_drain()
    _patch_multiwait_split()
    dt = mybir.dt
    nc = bass.Bass(num_devices=N_CORES)

    xT_ext = nc.declare_dram_parameter("xT", [D, T], dt.bfloat16, isOutput=False)
    wqT_ext = nc.declare_dram_parameter("wqT", [D, HD], dt.bfloat16, isOutput=False)
    wkT_ext = nc.declare_dram_parameter("wkT", [D, HD], dt.bfloat16, isOutput=False)
    wvT_ext = nc.declare_dram_parameter("wvT", [D, HD], dt.bfloat16, isOutput=False)
    wpT_ext = nc.declare_dram_parameter("wpT", [D, D], dt.bfloat16, isOutput=False)
    bias_ext = nc.declare_dram_parameter("bias", [1, D], dt.bfloat16, isOutput=False)
    out_ext = nc.declare_dram_parameter("out", [TS, D], dt.float32, isOutput=True)

    with tile.TileContext(nc) as tc, ExitStack() as ctx:
        singles = ctx.enter_context(tc.tile_pool(name="singles", bufs=1))
        dram = ctx.enter_context(tc.tile_pool(name="dram", bufs=2, space="DRAM"))
        cpool = ctx.enter_context(tc.tile_pool(name="cpool", bufs=4))

        # ---------- persistent SBUF ----------
        xT_k = [
            singles.tile([128, T], dt.bfloat16, tag=f"xT{k}", name=f"xT{k}")
            for k in range(KC)
        ]
        wqT_sb = singles.tile([128, KC, HD], dt.bfloat16)
        wkT_sb = singles.tile([128, KC, HD], dt.bfloat16)
        wvT_sb = singles.tile([128, KC, HD], dt.bfloat16)
        wpT_sb = singles.tile([128, KC, D], dt.bfloat16)
        bias_sb = singles.tile([1, D], dt.bfloat16)
        ones_sb = singles.tile([1, 128], dt.bfloat16)
        # qT/kT: partitions 0-63 = batch 0, 64-127 = batch 1; cols = token in batch
        # qT and v are split into small tiles so attention chunks can start
        # before the whole QKV phase finishes.
        qT_t = [
            singles.tile([128, 512], dt.bfloat16, tag=f"qT{i}", name=f"qT{i}")
            for i in range(N // 512)
        ]
        kT_t = [
            singles.tile([128, 512], dt.bfloat16, tag=f"kT{i}", name=f"kT{i}")
            for i in range(N // 512)
        ]
        vp_t = [
            singles.tile([128, 1 + HD], dt.bfloat16, tag=f"vp{t}", name=f"vp{t}")
            for t in range(T // 128)
        ]
        outTall_sb = singles.tile([128, KC, TS], dt.bfloat16)

        # Two half-token A2As: "a" carries the first 512 tokens of every
        # 1024-token shard, "b" the second 512 — so A2A "a" can fire while
        # the second half of attention still runs.
        HTS = TS // 2
        a2a_in_a = dram.tile([N_CORES, HD, HTS], dt.bfloat16)
        a2a_in_b = dram.tile([N_CORES, HD, HTS], dt.bfloat16)
        a2a_out_a = dram.tile([N_CORES, HD, HTS], dt.bfloat16)
        a2a_out_b = dram.tile([N_CORES, HD, HTS], dt.bfloat16)

        for k in range(KC):
            for c in range(4):
                nc.sync.dma_start(
                    out=xT_k[k][:, c * (T // 4):(c + 1) * (T // 4)],
                    in_=xT_ext[k * 128:(k + 1) * 128,
                               c * (T // 4):(c + 1) * (T // 4)],
                )
        for w_sb, w_ext in ((wqT_sb, wqT_ext), (wkT_sb, wkT_ext), (wvT_sb, wvT_ext)):
            nc.sync.dma_start(
                out=w_sb[:], in_=w_ext[:].rearrange("(k p) c -> p k c", p=128)
            )
        nc.sync.dma_start(
            out=wpT_sb[:], in_=wpT_ext[:].rearrange("(k p) c -> p k c", p=128)
        )
        nc.sync.dma_start(out=bias_sb[:], in_=bias_ext[:])
        nc.vector.memset(ones_sb[:], 1.0)
        for t in range(T // 128):
            nc.vector.memset(vp_t[t][:, 0:1], 1.0)

        # ---------- phase 1: kT then qT, both batches col-tiled concurrently --
        # k outermost + 8 persistent accumulators so matmuls on the first
        # xT chunk start as soon as its DMA lands. kT first: attention needs
        # all of kT but only the first qT tile to begin.
        def qk_phase(w_sb, copy_out, pname):
            with tc.tile_pool(name=pname, bufs=1, space="PSUM") as pqk:
                pss = [
                    pqk.tile([128, 512], dt.float32, name=f"{pname}{n}")
                    for n in range(N // 512)
                ]
                for k in range(KC):
                    for n in range(N // 512):
                        nc.tensor.matmul(
                            pss[n][0:64, :],
                            lhsT=w_sb[:, k, :],
                            rhs=xT_k[k][:, n * 512:(n + 1) * 512],
                            start=(k == 0),
                            stop=(k == KC - 1),
                            tile_position=(0, 0),
                        )
                        nc.tensor.matmul(
                            pss[n][64:128, :],
                            lhsT=w_sb[:, k, :],
                            rhs=xT_k[k][:, N + n * 512:N + (n + 1) * 512],
                            start=(k == 0),
                            stop=(k == KC - 1),
                            tile_position=(0, 64),
                        )
                for n in range(N // 512):
                    copy_out(n, pss[n])

        qk_phase(wkT_sb,
                 lambda n, ps: nc.vector.tensor_copy(kT_t[n][:], ps[:]), "pk")

        # ---------- phase 3: attention (both batches per iteration) ----------
        # qT tiles and v' tiles are produced through a single aux PSUM bank,
        # interleaved into the first q-chunk's kt loop so they ride in PE's
        # slack while ScalarE (the bottleneck) streams exps.
        with (
            tc.tile_pool(name="pst", bufs=2, space="PSUM") as pst,
            tc.tile_pool(name="pacc", bufs=4, space="PSUM") as pacc,
        ):
            def produce_qT(n):
                ps = pacc.tile([128, 512], dt.float32, tag="acc", name=f"pq{n}")
                for k in range(KC):
                    nc.tensor.matmul(
                        ps[0:64, :],
                        lhsT=wqT_sb[:, k, :],
                        rhs=xT_k[k][:, n * 512:(n + 1) * 512],
                        start=(k == 0), stop=(k == KC - 1),
                        tile_position=(0, 0),
                    )
                    nc.tensor.matmul(
                        ps[64:128, :],
                        lhsT=wqT_sb[:, k, :],
                        rhs=xT_k[k][:, N + n * 512:N + (n + 1) * 512],
                        start=(k == 0), stop=(k == KC - 1),
                        tile_position=(0, 64),
                    )
                nc.vector.tensor_copy(qT_t[n][:], ps[:])

            def produce_v(t):
                pv = pacc.tile([128, HD], dt.float32, tag="acc", name=f"pv{t}")
                for k in range(KC):
                    nc.tensor.matmul(
                        pv[:],
                        lhsT=xT_k[k][:, t * 128:(t + 1) * 128],
                        rhs=wvT_sb[:, k, :],
                        start=(k == 0), stop=(k == KC - 1),
                    )
                nc.vector.tensor_copy(vp_t[t][:, 1:1 + HD], pv[:])

            produce_qT(0)
            produce_v(0)
            produce_v(NKT)

            qc_order = [q for q in range(NQC) if q % 2 == 0] + \
                       [q for q in range(NQC) if q % 2 == 1]

            def emit_scores_exp(qc, kt):
                st = pst.tile([128, B, QC], dt.float32, tag="st",
                              name=f"st{qc}_{kt}")
                for pair in range(B):
                    pb = pair * 64
                    lhs_k = kT_t[kt // 4][pb:pb + 64,
                                          (kt % 4) * 128:(kt % 4) * 128 + 128]
                    nc.tensor.matmul(
                        st[:, pair, :],
                        lhsT=lhs_k,
                        rhs=qT_t[qc][pb:pb + 64, :],
                        start=True,
                        stop=True,
                        tile_position=(pb, 0),
                    )
                e = cpool.tile([128, B, QC], dt.bfloat16, tag="e", bufs=6,
                               name=f"e{qc}_{kt}")
                nc.scalar.activation(
                    e[:], st[:], mybir.ActivationFunctionType.Exp, scale=SCALE
                )
                return e

            se = emit_scores_exp(qc_order[0], 0)
            for qi, qc in enumerate(qc_order):
                accs = [
                    pacc.tile([1 + HD, QC], dt.float32, tag="acc", name=f"acc{qc}_{p}")
                    for p in range(B)
                ]
                for kt in range(NKT):
                    e = se
                    if kt < NKT - 1:
                        se = emit_scores_exp(qc, kt + 1)
                    elif qi + 1 < len(qc_order):
                        se = emit_scores_exp(qc_order[qi + 1], 0)
                    if qi == 0:
                        # pipeline the remaining qkv production into PE slack
                        if kt < NKT - 1:
                            produce_v(kt + 1)
                            produce_v(NKT + kt + 1)
                        if kt < NQC - 1:
                            produce_qT(kt + 1)
                    for pair in range(B):
                        vkt = vp_t[pair * NKT + kt][:]
                        nc.tensor.matmul(
                            accs[pair][:, :],
                            lhsT=vkt,
                            rhs=e[:, pair, :],
                            start=(kt == 0),
                            stop=(kt == NKT - 1),
                        )
                # normalization front half (DVE only): copy accumulator off
                # PSUM (frees the slot early), reciprocal of the denominator
                # row, cast for the matmul broadcast
                # normalize: copy accumulator off PSUM (frees the slot
                # early); reciprocal of the denominator row spread over 64
                # partitions via a strided DRAM bounce, then partition-
                # broadcast back via DRAM; everything off the PE/ACT streams
                for pair in range(B):
                    acc = accs[pair]
                    accS = cpool.tile([1 + HD, QC], dt.float32, tag="accS",
                                      name=f"accS{qc}_{pair}")
                    nc.vector.tensor_copy(accS[:], acc[:])
                    rdram = dram.tile([1, QC], dt.float32, tag="rdram")
                    nc.sync.dma_start(out=rdram[:], in_=accS[0:1, :])
                    spread = cpool.tile([64, QC // 64], dt.float32, tag="spread")
                    rap = rdram[:]
                    nc.sync.dma_start(
                        out=spread[:],
                        in_=bass.AP(
                            tensor=rap.tensor, offset=rap.offset,
                            ap=[[QC // 64, 64], [1, QC // 64]],
                        ),
                    )
                    rspread = cpool.tile([64, QC // 64], dt.float32, tag="rspread")
                    nc.vector.reciprocal(rspread[:], spread[:])
                    rdram2 = dram.tile([1, QC], dt.float32, tag="rdram2")
                    r2ap = rdram2[:]
                    nc.sync.dma_start(
                        out=bass.AP(
                            tensor=r2ap.tensor, offset=r2ap.offset,
                            ap=[[QC // 64, 64], [1, QC // 64]],
                        ),
                        in_=rspread[:],
                    )
                    bcast = cpool.tile([1 + HD, QC], dt.float32, tag="bcast")
                    nc.sync.dma_start(
                        out=bcast[:],
                        in_=bass.AP(
                            tensor=r2ap.tensor, offset=r2ap.offset,
                            ap=[[0, 1 + HD]] + list(r2ap.ap[1:]),
                        ),
                    )
                    outTn = cpool.tile([1 + HD, QC], dt.bfloat16, tag="outTn",
                                       name=f"oTn{qc}_{pair}")
                    nc.vector.tensor_mul(outTn[:], accS[:], bcast[:])
                    goff = pair * N + qc * QC      # global token offset
                    shard = goff // TS
                    half = a2a_in_a if (goff % TS) < HTS else a2a_in_b
                    nc.sync.dma_start(out=half[shard], in_=outTn[1:1 + HD, :])
                if qi == NQC // 2 - 1:
                    # all first-half shards written -> overlap this A2A with
                    # the remaining attention chunks
                    nc.gpsimd.collective_compute(
                        "AllToAll",
                        mybir.AluOpType.bypass,
                        replica_groups=[list(range(N_CORES))],
                        ins=[a2a_in_a.opt()],
                        outs=[a2a_out_a.opt()],
                    )
                    # Unpack on the (otherwise idle) GpSimd queue: the wait
                    # for A2A completion must NOT sit on the Sync queue, or
                    # it blocks the later normalization DMAs the attention
                    # loop depends on (observed 145us whole-chip stall).
                    for k in range(KC):
                        nc.gpsimd.dma_start(
                            out=outTall_sb[:, k, 0:HTS],
                            in_=a2a_out_a[2 * k:2 * k + 2].rearrange(
                                "a d n -> (a d) n"),
                        )

        # ---------- phase 4: output projection on own token slice ----------
        with tc.tile_pool(name="py", bufs=2, space="PSUM") as py:
            def proj_subtile(ts_i):
                yp = py.tile([128, D], dt.float32, name=f"yp{ts_i}", tag="yp")
                for k in range(KC):
                    nc.tensor.matmul(
                        yp[:],
                        lhsT=outTall_sb[:, k, ts_i * 128:(ts_i + 1) * 128],
                        rhs=wpT_sb[:, k, :],
                        start=(k == 0),
                        stop=False,
                    )
                nc.tensor.matmul(
                    yp[:],
                    lhsT=ones_sb[:],
                    rhs=bias_sb[:],
                    start=False,
                    stop=True,
                )
                y_sb = cpool.tile([128, D], dt.float32, tag="y", name=f"y{ts_i}")
                nc.vector.tensor_copy(y_sb[:], yp[:])
                nc.sync.dma_start(
                    out=out_ext[ts_i * 128:(ts_i + 1) * 128, :], in_=y_sb[:]
                )

            # first-half subtiles only need A2A "a" results -> can run while
            # A2A "b" is still in flight
            for ts_i in range(TS // 256):
                proj_subtile(ts_i)

            nc.gpsimd.collective_compute(
                "AllToAll",
                mybir.AluOpType.bypass,
                replica_groups=[list(range(N_CORES))],
                ins=[a2a_in_b.opt()],
                outs=[a2a_out_b.opt()],
            )
            for k in range(KC):
                nc.gpsimd.dma_start(
                    out=outTall_sb[:, k, HTS:TS],
                    in_=a2a_out_b[2 * k:2 * k + 2].rearrange("a d n -> (a d) n"),
                )
            for ts_i in range(TS // 256, TS // 128):
                proj_subtile(ts_i)

    return nc


def _get_nc():
    if "nc" not in _COMPILED:
        _COMPILED["nc"] = _build()
    return _COMPILED["nc"]


def kernel(x, w_qkv, w_proj, b_proj):
    from concourse.bass_utils import run_bass_kernel_spmd

    x = np.asarray(x, dtype=np.float32)
    w_qkv = np.asarray(w_qkv, dtype=np.float32)
    w_proj = np.asarray(w_proj, dtype=np.float32)
    b_proj = np.asarray(b_proj, dtype=np.float32)

    # host-side layout prep (bf16 compute precision on device)
    xT = np.ascontiguousarray(
        x.transpose(2, 0, 1).reshape(D, T)
    ).astype(BF16)
    wpT = np.ascontiguousarray(w_proj.T).astype(BF16)
    bias = b_proj.reshape(1, D).astype(BF16)

    in_maps = []
    for d in range(N_CORES):
        wq = w_qkv[0 * D + d * HD: 0 * D + (d + 1) * HD, :]   # [64, 512]
        wk = w_qkv[1 * D + d * HD: 1 * D + (d + 1) * HD, :]
        wv = w_qkv[2 * D + d * HD: 2 * D + (d + 1) * HD, :]
        in_maps.append({
            "xT": xT,
            "wqT": np.ascontiguousarray(wq.T).astype(BF16),
            "wkT": np.ascontiguousarray(wk.T).astype(BF16),
            "wvT": np.ascontiguousarray(wv.T).astype(BF16),
            "wpT": wpT,
            "bias": bias,
        })

    nc = _get_nc()
    res = run_bass_kernel_spmd(nc, in_maps, core_ids=list(range(N_CORES)))
    y = np.concatenate([res.results[d]["out"] for d in range(N_CORES)], axis=0)
    return y.reshape(B, N, D).astype(np.float32)



# revision 5
# speedup vs baseline: 1.4827x; 1.0337x over previous
"""Multi-head attention (B=2, N=4096, D=512, H=8) on 8 TRN2 NeuronCores.

Sharding: head-parallel (core d owns head d, both batches). v2 layout:
  - Token-major streamed xT DMA: fine-grained pieces for the first two
    512-token blocks so kT[0]/qT[0] matmuls start ~5us in; first exp ~10us.
  - JIT production: kT/qT/v tiles are produced inside the attention loop's
    PE slack, gated on their xT blocks' DMA arrival.
  - PV matmuls run on a quota-paced lag behind the exp stream (per-qc pair
    quotas) so the production-heavy first q-chunks don't starve ScalarE,
    which is the bottleneck (33.5M exps/core ~ 287us busy).
  - Output A2A split into 3 pipelined pieces (after qc3 / qc5 / qc7) with
    token ownership striped across completion order; unpack DMAs + their
    waits ride the idle GpSimd queue (never block the Sync queue), proj
    subtiles for pieces 0-1 are interleaved into late attention.
  - fp16 everywhere off-PSUM (better mantissa than bf16; same speed).
Host side only transposes/casts inputs and scatters the 8 output slices.
"""

from collections import deque
from contextlib import ExitStack

import numpy as np

N_CORES = 8
B, N, D = 2, 4096, 512
H, HD = 8, 64
T = B * N              # 8192 flattened tokens
TS = T // N_CORES      # 1024 tokens output slice per core
SCALE = HD ** -0.5
KC = D // 128          # 4 contraction chunks of the model dim
NKT = N // 128         # 32 k-token tiles per batch
QC = 512               # q-chunk processed per accumulator
NQC = N // QC          # 8 q-chunks per batch

F16 = np.float16

# PV emission quota per qc, in (kt, both-batch) pair units; sums to 256.
# Lag after each qc: 16, 22, 18, 8, 0, 0, 0, 0 — smooths the JIT
# production load of qc0-2 into later windows, drains before the tail.
PV_QUOTA = [16, 26, 36, 42, 40, 32, 32, 32]

_COMPILED = {}


def _patch_tile_drain():
    """The walrus build in this container caps sync waits at 1 per
    instruction (2 for EventSemaphore), but TileContext._drain_and_barrier
    puts every live proc's final wait on a single Drain, which fails
    codegen with 'Too many sync wait commands'. Re-emit those waits as
    individual wait_ge instructions before the drain."""
    import concourse.mybir as mybir
    import concourse.tile as tile
    from concourse.bass_types import SemaphoreHandle
    from concourse.vector_clock import ScopedClock

    if getattr(tile.TileContext, "_drain_patch_installed", False):
        return

    def _drain_and_barrier(self, tick_clock, wait_clock):
        probe = mybir.InstNoOp(name=f"drain-probe-{self.nc.next_id()}", ins=[], outs=[])
        probe.engine = mybir.EngineType.SP
        wait_clock.add_sem_waits(probe, ScopedClock({None: tick_clock.global_clock}))
        waits = probe.sync_info.on_wait if probe.sync_info is not None else []
        for w in waits:
            assert w.wait_mode == "sem-ge-imm", w
            self.nc.sync.wait_ge(SemaphoreHandle(w.ant_name, w.id), w.wait_value)
        self.nc.sync.drain()

        self.nc.all_engine_barrier()
        assert self.sems is not None
        popped = self.nc._tile_sem_poison_stack.pop()
        assert popped is self._sem_poison
        self.nc.clear_and_free_semaphores(list(self.sems.allocated().values()))
        self.nc.all_engine_barrier()

    tile.TileContext._drain_and_barrier = _drain_and_barrier
    tile.TileContext._drain_patch_installed = True


def _patch_multiwait_split():
    """This walrus build rejects instructions with more than one sync wait
    ('Too many sync wait commands'), but Tile's wait assigner can emit
    several waits on one instruction. Post-process the serialized BIR:
    move excess waits onto single-wait EventSemaphore instructions inserted
    just before the owning instruction (same engine => executes in order)."""
    import json

    import concourse.bass as bass

    if getattr(bass.Bass, "_multiwait_patch_installed", False):
        return
    orig = bass.Bass.to_json_bytes

    def to_json_bytes(self, *a, **kw):
        data = json.loads(orig(self, *a, **kw))
        n_split = 0
        for fn in data.get("functions", []):
            for bb in fn.get("blocks", []):
                insts = bb.get("instructions")
                if not insts:
                    continue
                out = []
                for inst in insts:
                    si = inst.get("sync_info")
                    ow = (si or {}).get("on_wait") or []
                    if len(ow) > 1:
                        for i, w in enumerate(ow[:-1]):
                            out.append({
                                "debug": inst.get("debug", 0),
                                "engine": inst["engine"],
                                "ins": [],
                                "outs": [],
                                "name": f"{inst['name']}-esw{i}",
                                "opcode": "EventSemaphore",
                                "sync_info": {"on_update": [], "on_wait": [w]},
                            })
                            n_split += 1
                        si["on_wait"] = [ow[-1]]
                    out.append(inst)
                bb["instructions"] = out
        return json.dumps(data).encode()

    bass.Bass.to_json_bytes = to_json_bytes
    bass.Bass._multiwait_patch_installed = True


def _build():
    import concourse.bass as bass
    import concourse.mybir as mybir
    import concourse.tile as tile

    _patch_tile_drain()
    _patch_multiwait_split()
    dt = mybir.dt
    nc = bass.Bass(num_devices=N_CORES)

    xT_ext = nc.declare_dram_parameter("xT", [D, T], dt.float16, isOutput=False)
    wqT_ext = nc.declare_dram_parameter("wqT", [D, HD], dt.float16, isOutput=False)
    wkT_ext = nc.declare_dram_parameter("wkT", [D, HD], dt.float16, isOutput=False)
    wvT_ext = nc.declare_dram_parameter("wvT", [D, HD], dt.float16, isOutput=False)
    wpT_ext = nc.declare_dram_parameter("wpT", [D, D], dt.float16, isOutput=False)
    bias_ext = nc.declare_dram_parameter("bias", [1, D], dt.float16, isOutput=False)
    out_ext = nc.declare_dram_parameter("out", [TS, D], dt.float16, isOutput=True)

    with tile.TileContext(nc) as tc, ExitStack() as ctx:
        singles = ctx.enter_context(tc.tile_pool(name="singles", bufs=1))
        dram = ctx.enter_context(tc.tile_pool(name="dram", bufs=4, space="DRAM"))
        cpool = ctx.enter_context(tc.tile_pool(name="cpool", bufs=4))

        # ---------- persistent SBUF ----------
        xT_k = [
            singles.tile([128, T], dt.float16, tag=f"xT{k}", name=f"xT{k}")
            for k in range(KC)
        ]
        wqT_sb = singles.tile([128, KC, HD], dt.float16)
        wkT_sb = singles.tile([128, KC, HD], dt.float16)
        wvT_sb = singles.tile([128, KC, HD], dt.float16)
        wpT_sb = singles.tile([128, KC, D], dt.float16)
        bias_sb = singles.tile([1, D], dt.float16)
        ones_sb = singles.tile([1, 128], dt.float16)
        kT_t = [
            singles.tile([128, 512], dt.float16, tag=f"kT{i}", name=f"kT{i}")
            for i in range(NQC)
        ]
        qT_t = [
            singles.tile([128, 512], dt.float16, tag=f"qT{i}", name=f"qT{i}")
            for i in range(NQC)
        ]
        vp_t = [
            singles.tile([128, 1 + HD], dt.float16, tag=f"vp{t}", name=f"vp{t}")
            for t in range(T // 128)
        ]
        outTall_sb = singles.tile([128, KC, TS], dt.float16)

        # A2A pieces: P0 after qc3 (512 tok/dest), P1 after qc5 (256),
        # P2 after qc7 (256). dim0 = dest core for in, src core for out.
        a2a_in = [
            dram.tile([N_CORES, HD, n], dt.float16, tag=f"a2a_in{p}", bufs=1,
                      name=f"a2a_in{p}")
            for p, n in ((0, 512), (1, 256), (2, 256))
        ]
        a2a_out = [
            dram.tile([N_CORES, HD, n], dt.float16, tag=f"a2a_out{p}", bufs=1,
                      name=f"a2a_out{p}")
            for p, n in ((0, 512), (1, 256), (2, 256))
        ]
        # outTall col ranges fed by each piece
        piece_cols = [(0, 512), (512, 768), (768, 1024)]

        # exp table warm-up: a dummy activation with no data deps loads the
        # ACT table set (~2.7us) during the DMA window instead of at the
        # first real exp.
        warm_in = singles.tile([1, 16], dt.float32)
        warm_out = singles.tile([1, 16], dt.float16)
        nc.vector.memset(warm_in[:], 0.0)
        nc.scalar.activation(
            warm_out[:], warm_in[:], mybir.ActivationFunctionType.Exp
        )

        # ---------- weights + constants ----------
        for w_sb, w_ext in ((wqT_sb, wqT_ext), (wkT_sb, wkT_ext), (wvT_sb, wvT_ext)):
            nc.sync.dma_start(
                out=w_sb[:], in_=w_ext[:].rearrange("(k p) c -> p k c", p=128)
            )
        nc.sync.dma_start(
            out=wpT_sb[:], in_=wpT_ext[:].rearrange("(k p) c -> p k c", p=128)
        )
        nc.sync.dma_start(out=bias_sb[:], in_=bias_ext[:])
        nc.vector.memset(ones_sb[:], 1.0)
        for t in range(T // 128):
            nc.vector.memset(vp_t[t][:, 0:1], 1.0)

        # ---------- xT DMA, token-major ----------
        # blocks 0-1: one DMA per (blk, b, k) -> 16 small pieces spread
        # across queues so block 0 lands ~5us in. blocks 2-7: one DMA per
        # (blk, k) covering both batches via a strided free dim.
        for blk in range(2):
            for b in range(B):
                for k in range(KC):
                    c0 = b * N + blk * 512
                    nc.sync.dma_start(
                        out=xT_k[k][:, c0:c0 + 512],
                        in_=xT_ext[k * 128:(k + 1) * 128, c0:c0 + 512],
                    )
        for blk in range(2, NQC):
            for k in range(KC):
                t_ap = xT_k[k][:]
                e_ap = xT_ext[k * 128:(k + 1) * 128, :]
                off = blk * 512
                nc.sync.dma_start(
                    out=bass.AP(
                        tensor=t_ap.tensor, offset=t_ap.offset + off,
                        ap=[list(t_ap.ap[0]), [N, B], [1, 512]],
                    ),
                    in_=bass.AP(
                        tensor=e_ap.tensor, offset=e_ap.offset + off,
                        ap=[list(e_ap.ap[0]), [N, B], [1, 512]],
                    ),
                )

        # ---------- attention ----------
        with (
            tc.tile_pool(name="pst", bufs=2, space="PSUM") as pst,
            tc.tile_pool(name="pacc", bufs=4, space="PSUM") as pacc,
        ):
            def produce_kq(w_sb, dst, blk, pname):
                ps = pacc.tile([128, 512], dt.float32, tag="acc", name=f"{pname}{blk}")
                for k in range(KC):
                    nc.tensor.matmul(
                        ps[0:64, :],
                        lhsT=w_sb[:, k, :],
                        rhs=xT_k[k][:, blk * 512:(blk + 1) * 512],
                        start=(k == 0), stop=(k == KC - 1),
                        tile_position=(0, 0),
                    )
                    nc.tensor.matmul(
                        ps[64:128, :],
                        lhsT=w_sb[:, k, :],
                        rhs=xT_k[k][:, N + blk * 512:N + (blk + 1) * 512],
                        start=(k == 0), stop=(k == KC - 1),
                        tile_position=(0, 64),
                    )
                nc.vector.tensor_copy(dst[:], ps[:])

            def produce_v(t):
                pv = pacc.tile([128, HD], dt.float32, tag="acc", name=f"pv{t}")
                for k in range(KC):
                    nc.tensor.matmul(
                        pv[:],
                        lhsT=xT_k[k][:, t * 128:(t + 1) * 128],
                        rhs=wvT_sb[:, k, :],
                        start=(k == 0), stop=(k == KC - 1),
                    )
                nc.vector.tensor_copy(vp_t[t][:, 1:1 + HD], pv[:])

            def emit_scores_exp(qc, kt):
                st = pst.tile([128, B, QC], dt.float32, tag="st",
                              name=f"st{qc}_{kt}")
                for pair in range(B):
                    pb = pair * 64
                    lhs_k = kT_t[kt // 4][pb:pb + 64,
                                          (kt % 4) * 128:(kt % 4) * 128 + 128]
                    nc.tensor.matmul(
                        st[:, pair, :],
                        lhsT=lhs_k,
                        rhs=qT_t[qc][pb:pb + 64, :],
                        start=True,
                        stop=True,
                        tile_position=(pb, 0),
                    )
                e = cpool.tile([128, B, QC], dt.float16, tag="e", bufs=26,
                               name=f"e{qc}_{kt}")
                nc.scalar.activation(
                    e[:], st[:], mybir.ActivationFunctionType.Exp, scale=SCALE
                )
                return e

            # norm chain: reciprocal of the denominator row, DRAM-bounce
            # partition broadcast, one fused scale-multiply off PSUM, then
            # the A2A slice writes.
            def emit_norm(qc, accs):
                for pair in range(B):
                    acc = accs[pair]
                    rvec = cpool.tile([1, QC], dt.float32, tag="rvec",
                                      name=f"rv{qc}_{pair}")
                    nc.vector.reciprocal(rvec[:], acc[0:1, :])
                    rdram = dram.tile([1, QC], dt.float32, tag="rdram")
                    nc.sync.dma_start(out=rdram[:], in_=rvec[:])
                    bcast = cpool.tile([1 + HD, QC], dt.float32, tag="bcast")
                    r_ap = rdram[:]
                    nc.sync.dma_start(
                        out=bcast[:],
                        in_=bass.AP(
                            tensor=r_ap.tensor, offset=r_ap.offset,
                            ap=[[0, 1 + HD]] + list(r_ap.ap[1:]),
                        ),
                    )
                    # PSUM reads must start partition-aligned: multiply the
                    # full [65, QC] accumulator, slice the SBUF result.
                    outTn = cpool.tile([1 + HD, QC], dt.float16, tag="outTn",
                                       name=f"oTn{qc}_{pair}")
                    nc.vector.tensor_mul(outTn[:], acc[:, :], bcast[:])
                    # scatter the two 256-token halves into the A2A piece
                    if qc < 4:
                        piece, j0 = 0, qc * 4 + pair * 2
                    elif qc < 6:
                        piece, j0 = 1, (qc - 4) * 4 + pair * 2
                    else:
                        piece, j0 = 2, (qc - 6) * 4 + pair * 2
                    for h in range(2):
                        j = j0 + h
                        dest, pos = j % N_CORES, j // N_CORES
                        nc.sync.dma_start(
                            out=a2a_in[piece][dest][:, pos * 256:pos * 256 + 256],
                            in_=outTn[1:1 + HD, h * 256:h * 256 + 256],
                        )

            def trigger_piece(p):
                nc.gpsimd.collective_compute(
                    "AllToAll",
                    mybir.AluOpType.bypass,
                    replica_groups=[list(range(N_CORES))],
                    ins=[a2a_in[p].opt()],
                    outs=[a2a_out[p].opt()],
                )

            def unpack_piece(p):
                lo, hi = piece_cols[p]
                for k in range(KC):
                    nc.gpsimd.dma_start(
                        out=outTall_sb[:, k, lo:hi],
                        in_=a2a_out[p][2 * k:2 * k + 2].rearrange(
                            "a d n -> (a d) n"),
                    )

            def proj_subtile(ts_i):
                yp = pacc.tile([128, D], dt.float32, tag="acc", name=f"yp{ts_i}")
                for k in range(KC):
                    nc.tensor.matmul(
                        yp[:],
                        lhsT=outTall_sb[:, k, ts_i * 128:(ts_i + 1) * 128],
                        rhs=wpT_sb[:, k, :],
                        start=(k == 0),
                        stop=False,
                    )
                nc.tensor.matmul(
                    yp[:],
                    lhsT=ones_sb[:],
                    rhs=bias_sb[:],
                    start=False,
                    stop=True,
                )
                y_sb = cpool.tile([128, D], dt.float16, tag="y", name=f"y{ts_i}")
                nc.vector.tensor_copy(y_sb[:], yp[:])
                nc.sync.dma_start(
                    out=out_ext[ts_i * 128:(ts_i + 1) * 128, :], in_=y_sb[:]
                )

            # pre-loop production (gated on block-0/1 DMA + weights)
            produce_kq(wkT_sb, kT_t[0], 0, "k")
            produce_kq(wqT_sb, qT_t[0], 0, "q")
            produce_kq(wkT_sb, kT_t[1], 1, "k")

            # JIT production schedule: extras[(qc, kt)] emitted right after
            # that slot's exp.
            extras = {}
            for blk in range(2, NQC):
                extras.setdefault((0, 4 * blk - 6), []).append(
                    (lambda b: lambda: produce_kq(wkT_sb, kT_t[b], b, "k"))(blk))
            extras.setdefault((0, 26), []).append(
                lambda: produce_kq(wqT_sb, qT_t[1], 1, "q"))
            for i, n_ in enumerate(range(2, 6)):
                extras.setdefault((1, 2 + 8 * i), []).append(
                    (lambda m: lambda: produce_kq(wqT_sb, qT_t[m], m, "q"))(n_))
            for i, n_ in enumerate(range(6, NQC)):
                extras.setdefault((2, 2 + 8 * i), []).append(
                    (lambda m: lambda: produce_kq(wqT_sb, qT_t[m], m, "q"))(n_))

            # PV stream state
            pending = deque()   # (qc, kt, e_tile)
            acc_of = {}         # qc -> [acc_b0, acc_b1]
            v_done = set()
            pv_emitted = 0

            def emit_pv_pair():
                qc, kt, e = pending.popleft()
                if qc not in acc_of:
                    acc_of[qc] = [
                        pacc.tile([1 + HD, QC], dt.float32, tag="acc",
                                  name=f"acc{qc}_{p}")
                        for p in range(B)
                    ]
                for pair in range(B):
                    vidx = pair * NKT + kt
                    if vidx not in v_done:
                        produce_v(vidx)
                        v_done.add(vidx)
                    nc.tensor.matmul(
                        acc_of[qc][pair][:, :],
                        lhsT=vp_t[vidx][:],
                        rhs=e[:, pair, :],
                        start=(kt == 0),
                        stop=(kt == NKT - 1),
                    )
                if kt == NKT - 1:
                    emit_norm(qc, acc_of.pop(qc))
                    if qc == 3:
                        trigger_piece(0)
                    elif qc == 5:
                        trigger_piece(1)
                    elif qc == 7:
                        trigger_piece(2)

            for qc in range(NQC):
                base = pv_emitted
                for kt in range(NKT):
                    e = emit_scores_exp(qc, kt)
                    pending.append((qc, kt, e))
                    for fn in extras.get((qc, kt), ()):
                        fn()
                    # pace PV emission: quota spread evenly across the qc
                    goal = base + (PV_QUOTA[qc] * (kt + 1)) // NKT
                    while pv_emitted < goal and pending:
                        emit_pv_pair()
                        pv_emitted += 1
                # mid-attention unpack + proj overlap
                if qc == 5:
                    unpack_piece(0)
                    for ts_i in range(4):
                        proj_subtile(ts_i)

            # drain remaining PVs (norm qc7 + trigger P2 fire from inside)
            while pending:
                emit_pv_pair()
                pv_emitted += 1

            unpack_piece(1)
            for ts_i in range(4, 6):
                proj_subtile(ts_i)
            unpack_piece(2)
            for ts_i in range(6, 8):
                proj_subtile(ts_i)

    return nc


def _get_nc():
    if "nc" not in _COMPILED:
        _COMPILED["nc"] = _build()
    return _COMPILED["nc"]


def _seg_token(piece, j):
    """Map (piece, subchunk index) -> (batch, token start). 256-token segs."""
    if piece == 0:
        qc = j // 4
    elif piece == 1:
        qc = 4 + j // 4
    else:
        qc = 6 + j // 4
    rem = j % 4
    b, h = rem // 2, rem % 2
    return b, qc * 512 + h * 256


def kernel(x, w_qkv, w_proj, b_proj):
    from concourse.bass_utils import run_bass_kernel_spmd

    x = np.asarray(x, dtype=np.float32)
    w_qkv = np.asarray(w_qkv, dtype=np.float32)
    w_proj = np.asarray(w_proj, dtype=np.float32)
    b_proj = np.asarray(b_proj, dtype=np.float32)

    xT = np.ascontiguousarray(x.transpose(2, 0, 1).reshape(D, T)).astype(F16)
    wpT = np.ascontiguousarray(w_proj.T).astype(F16)
    bias = b_proj.reshape(1, D).astype(F16)

    in_maps = []
    for d in range(N_CORES):
        wq = w_qkv[0 * D + d * HD: 0 * D + (d + 1) * HD, :]
        wk = w_qkv[1 * D + d * HD: 1 * D + (d + 1) * HD, :]
        wv = w_qkv[2 * D + d * HD: 2 * D + (d + 1) * HD, :]
        in_maps.append({
            "xT": xT,
            "wqT": np.ascontiguousarray(wq.T).astype(F16),
            "wkT": np.ascontiguousarray(wk.T).astype(F16),
            "wvT": np.ascontiguousarray(wv.T).astype(F16),
            "wpT": wpT,
            "bias": bias,
        })

    nc = _get_nc()
    res = run_bass_kernel_spmd(nc, in_maps, core_ids=list(range(N_CORES)))

    y = np.empty((B, N, D), dtype=np.float32)
    for s in range(N_CORES):
        r = np.asarray(res.results[s]["out"], dtype=np.float32)
        segs = [(0, s), (0, s + 8), (1, s), (2, s)]
        for i, (piece, j) in enumerate(segs):
            b, t0 = _seg_token(piece, j)
            y[b, t0:t0 + 256, :] = r[i * 256:(i + 1) * 256, :]
    return y
